# revision 32
# baseline (speedup 1.0000x reference)
import sys
import numpy as np

for _p in ("/opt/trn_rl_repo", "/root/.axon_site/_ro/trn_rl_repo"):
    if _p not in sys.path:
        sys.path.append(_p)

import os
import ml_dtypes

BF = ml_dtypes.bfloat16

B, N, NODE, FE = 128, 100, 2, 128
NODE_SIZES = [2, 16, 32]
FN2_OUT = [14, 30, 1]
NCORES = 8
GPC = B // NCORES            # graphs per core = 16
COLS = GPC * N               # 1600
PAIRS = N * N                # 10000
UPG = 10                     # units (1000-col) per graph
CPB = GPC * UPG * 2          # 500-col chunks per block = 320

# Engine split fractions. GPSIMD cannot touch PSUM on real hardware, so the
# t1 lrelu (reads PSUM) runs on DVE ('D') or ACT ('A') at unit granularity;
# the SBUF-only tree adds are what Pool can take.
UPB = GPC * UPG                                      # units per block = 160
FRAC_A = float(os.environ.get("K_FRAC_A", 20 / 160))  # t1 units on ACT (rest DVE)
FRAC_L1D = float(os.environ.get("K_FRAC_L1D", 0.0))   # l1 units on DVE (rest Pool)
FRAC_L2P = float(os.environ.get("K_FRAC_L2P", 24 / 160))  # l2 units on Pool (rest DVE)
STT_DELAY = int(os.environ.get("K_STT_DELAY", 1))    # units
FE2_DELAY = int(os.environ.get("K_FE2_DELAY", 2))    # units
TREE_DELAY = int(os.environ.get("K_TREE_DELAY", 4))  # units


def _frac_assign(frac, mark, other, n=UPB):
    out = []
    acc = 0.0
    for _ in range(n):
        acc += frac
        if acc >= 1.0:
            out.append(mark)
            acc -= 1.0
        else:
            out.append(other)
    return out


def round_fp32r(a):
    u = np.ascontiguousarray(np.asarray(a, np.float32)).view(np.uint32)
    low = u & np.uint32(0xFFF)
    base = u & np.uint32(0xFFFFF000)
    add = ((low > 0x800) | ((low == 0x800) & (((u >> 12) & 1) == 1))).astype(np.uint32) << 12
    return (base + add).view(np.float32)


_CACHE = {}

# ---- packed constant layouts (shared between _build and _host_prep) ----
# bf16 pack: [128 rows, ...cols]: per block: s1 (2d+2 rows x 128), fe2T (128x128),
# fn1xT (d x 128)
def _bf_pack_layout():
    cols = {}
    c = 0
    for i in range(3):
        d = NODE_SIZES[i]
        od = FN2_OUT[i]
        cols[f"s1_{i}"] = (2 * d + 2, c, 128); c += 128
        cols[f"fe2T{i}"] = (128, c, 128); c += 128
        cols[f"fn1xT{i}"] = (d, c, 128); c += 128
        cols[f"fn1avT{i}"] = (128, c, 128); c += 128
        cols[f"fn2T{i}"] = (128, c, od); c += od
    return cols, c


def _f32_pack_layout():
    cols = {}
    c = 0
    for i in range(3):
        od = FN2_OUT[i]
        cols[f"b2_{i}"] = (128, c, 1); c += 1
        cols[f"fb1_{i}"] = (128, c, 1); c += 1
        cols[f"fb2_{i}"] = (od, c, 1); c += 1
    return cols, c


def _fn_pair(nc, g0, Qbuf, xt, xnext, ty, d, od, fn1avT, fn1xT, fn2T, fb1, fb2,
             ps1, ps2, y1pool, F32, BF16, AF, blk):
    """fn stage for graphs g0, g0+1 (200 node columns). The av j-sum tail is
    folded into the fn1av matmul: Qbuf holds 25 partial-sum columns per node
    (quarter-reduced av2), summed here via 25 accumulating matmuls."""
    csl = slice(g0 * N, (g0 + 2) * N)
    pf = ps1.tile([128, 512], F32, tag="p1")
    Qv = Qbuf[:].rearrange("p (a b) -> p a b", b=25)       # [128, 200, 25]
    for c in range(25):
        nc.tensor.matmul(pf[:, 0:200], fn1avT, Qv[:, :, c:c + 1],
                         start=(c == 0), stop=False)
    nc.tensor.matmul(pf[:, 0:200], fn1xT, xt[0:d, csl], start=False, stop=True)
    y1 = y1pool.tile([128, 200], BF16, tag="y1")
    nc.scalar.activation(y1[:], pf[:, 0:200], AF.Tanh, bias=fb1)
    pf2 = ps2.tile([od, 512], F32, tag="p2")
    nc.tensor.matmul(pf2[:, 0:200], fn2T, y1[:], start=True, stop=True)
    if blk < 2:
        nc.scalar.activation(xnext[0:od, csl], pf2[:, 0:200], AF.Tanh, bias=fb2)
    else:
        nc.scalar.activation(ty[:, csl], pf2[:, 0:200], AF.Tanh, bias=fb2)


def _build():
    import concourse.bacc as bacc
    import concourse.mybir as mybir
    import concourse.tile as tile

    F32 = mybir.dt.float32
    F32R = mybir.dt.float32r
    BF16 = mybir.dt.bfloat16
    AF = mybir.ActivationFunctionType
    ALU = mybir.AluOpType
    AX = mybir.AxisListType

    nc = bacc.Bacc("TRN2", target_bir_lowering=False, debug=False, num_devices=NCORES)

    bfl, nbf = _bf_pack_layout()
    f32l, nf32 = _f32_pack_layout()

    din = {}
    din["xt0"] = nc.dram_tensor("xt0", [3, COLS], BF16, kind="ExternalInput")
    din["nrmo"] = nc.dram_tensor("nrmo", [2 * GPC, PAIRS], BF16, kind="ExternalInput")
    din["bfpack"] = nc.dram_tensor("bfpack", [128, nbf], BF16, kind="ExternalInput")
    din["f32pack"] = nc.dram_tensor("f32pack", [128, nf32], F32, kind="ExternalInput")
    out_d = nc.dram_tensor("out", [1, GPC], F32, kind="ExternalOutput")

    T1A = _frac_assign(FRAC_A, "A", "D")
    L1A = _frac_assign(FRAC_L1D, "D", "P")
    L2A = _frac_assign(FRAC_L2P, "P", "D")

    with tile.TileContext(nc) as tc:
        with (
            tc.tile_pool(name="const", bufs=1) as cpool,
            tc.tile_pool(name="xp", bufs=1) as xpool,
            tc.tile_pool(name="m2", bufs=3) as mpool,
            tc.tile_pool(name="rep", bufs=3) as rpool,
            tc.tile_pool(name="t1p", bufs=8) as tpool,
            tc.tile_pool(name="avp", bufs=8) as apool,
            tc.tile_pool(name="q1p", bufs=3) as q1pool,
            tc.tile_pool(name="qb", bufs=3) as qbpool,
            tc.tile_pool(name="y1p", bufs=2) as y1pool,
            tc.tile_pool(name="ps1", bufs=2, space="PSUM") as ps1,
            tc.tile_pool(name="ps2", bufs=2, space="PSUM") as ps2,
        ):
            XT0 = cpool.tile([3, COLS], BF16, tag="xt0", name="w_xt0")
            nc.sync.dma_start(XT0[:], din["xt0"].ap())
            BFP = cpool.tile([128, nbf], BF16, tag="bfp", name="w_bfp")
            nc.sync.dma_start(BFP[:], din["bfpack"].ap())
            # f32pack is only needed by the fn stage; load it after the
            # first graph's M2 DMAs are queued (SP queue is FIFO).
            F32P = cpool.tile([128, nf32], F32, tag="f32p", name="w_f32p")
            deferred_loads = [
                (F32P, din["f32pack"]),
            ]

            def Wv(name):
                if name in bfl:
                    r, c, w = bfl[name]
                    return BFP[0:r, c:c + w]
                r, c, w = f32l[name]
                return F32P[0:r, c:c + w]

            xt = XT0                        # [3, COLS] block-0 x (+ones row)
            ty = None
            for blk in range(3):
                d = NODE_SIZES[blk]
                od = FN2_OUT[blk]
                s1 = Wv(f"s1_{blk}")
                fe2T = Wv(f"fe2T{blk}")
                b2 = Wv(f"b2_{blk}")
                fn1avT, fn1xT = Wv(f"fn1avT{blk}"), Wv(f"fn1xT{blk}")
                fn2T = Wv(f"fn2T{blk}")
                fb1, fb2 = Wv(f"fb1_{blk}"), Wv(f"fb2_{blk}")
                # fe2/av2 are emitted 2 units late and tree (j-sum) work 4
                # units later still: every engine's in-order queue then only
                # sees instructions whose inputs were produced units ago, so
                # no engine parks waiting on a just-issued producer.
                pending_fe2 = []
                pending_trees = []
                pending_stt = []
                qbufs = {}                   # graph-pair -> Qbuf tile

                def emit_stt(p1_, t1_, ku_):
                    p1v = p1_[:].rearrange("p (a b) -> p a b", b=512)[:, :, 0:500]
                    t1v = t1_[:].rearrange("p (a b) -> p a b", b=500)
                    if T1A[ku_ % UPB] == "A":
                        nc.scalar.activation(t1v, p1v, AF.Prelu,
                                             scale=1.0, alpha=0.2)
                    else:
                        nc.vector.scalar_tensor_tensor(t1v, p1v, 0.2, p1v,
                                                       ALU.mult, ALU.max)

                def emit_fe2(t1_, g_, u_):
                    p2 = ps2.tile([128, 1024], F32, tag="p2")
                    for ci in range(2):
                        nc.tensor.matmul(p2[:, ci * 512: ci * 512 + 500],
                                         fe2T, t1_[:, ci * 500:(ci + 1) * 500],
                                         start=True, stop=True)
                    p2v = p2[:].rearrange("p (a b) -> p a b", b=512)[:, :, 0:500]
                    av2 = apool.tile([128, 1000], BF16, tag="av2")
                    av2v = av2[:].rearrange("p (a b) -> p a b", b=500)
                    nc.scalar.activation(av2v, p2v, AF.Prelu,
                                         bias=b2, scale=1.0, alpha=0.2)
                    pending_trees.append((av2, g_, u_))
                    if len(pending_trees) > TREE_DELAY:
                        emit_tree(*pending_trees.pop(0))

                def emit_tree(av2_, g_, u_):
                    # two bf16 half-adds (SBUF-only, so Pool may take them);
                    # the remaining 25-way sum per node is folded into the
                    # fn1av matmul (see _fn_pair).
                    ku = g_ * UPG + u_
                    a3 = av2_[:].rearrange("p (a b) -> p a b", b=N)
                    q1 = q1pool.tile([128, 500], BF16, tag="q1")
                    q1v = q1[:].rearrange("p (a b) -> p a b", b=50)
                    e1 = nc.vector if L1A[ku % UPB] == "D" else nc.gpsimd
                    e1.tensor_tensor(q1v, a3[:, :, 0:50], a3[:, :, 50:100],
                                     ALU.add)
                    pr = g_ // 2
                    if pr not in qbufs:
                        qbufs[pr] = qbpool.tile([128, 5000], BF16, tag="qb", name=f"qb{pr % 3}")
                    base = (g_ % 2) * 2500 + u_ * 250
                    q2v = qbufs[pr][:, base:base + 250].rearrange(
                        "p (a b) -> p a b", b=25)
                    e2 = nc.gpsimd if L2A[ku % UPB] == "P" else nc.vector
                    e2.tensor_tensor(q2v, q1v[:, :, 0:25], q1v[:, :, 25:50],
                                     ALU.add)

                if blk < 2:
                    nd = NODE_SIZES[blk + 1]
                    nod = FN2_OUT[blk]
                    xnext = xpool.tile([nd + 1, COLS], BF16, tag=f"x{blk + 1}")
                    # coords + ones rows below the y rows, ready immediately
                    nc.sync.dma_start(xnext[nod:nod + 3, :], XT0[:, :])
                else:
                    ty = xpool.tile([1, COLS], F32, tag="ty")

                for g in range(GPC):
                    gsl = slice(g * N, (g + 1) * N)
                    # ---- build M2 = [x1; x2; nrm; ones]  [2d+2, PAIRS] bf16
                    rep4 = rpool.tile([d, 400], BF16, tag="rep4")
                    nc.sync.dma_start(
                        rep4[:].rearrange("p (a b) -> p a b", a=4),
                        xt[0:d, gsl].unsqueeze(1).broadcast_to([d, 4, N]),
                    )
                    M2 = mpool.tile([2 * d + 2, PAIRS], BF16, tag="m2")
                    nc.sync.dma_start(
                        M2[0:d, :].rearrange("p (a b) -> p a b", b=N),
                        xt[0:d, gsl].unsqueeze(2).broadcast_to([d, N, N]),
                    )
                    nc.sync.dma_start(
                        M2[d:2 * d, :].rearrange("p (a b) -> p a b", a=25),
                        rep4[:].unsqueeze(1).broadcast_to([d, 25, 400]),
                    )
                    nc.sync.dma_start(
                        M2[2 * d:2 * d + 2, :],
                        din["nrmo"].ap()[2 * g:2 * g + 2, :],
                    )
                    while deferred_loads:
                        tile_, dram_ = deferred_loads.pop(0)
                        nc.sync.dma_start(tile_[:], dram_.ap())

                    for u in range(UPG):
                        t1 = tpool.tile([128, 1000], BF16, tag="t1")
                        # ---- fe1 (bias baked in); the t1 lrelu is emitted
                        # late so no engine's queue head ever waits on a
                        # just-issued matmul.
                        p1 = ps1.tile([128, 1024], F32, tag="p1")
                        for ci in range(2):
                            c0 = u * 1000 + ci * 500
                            nc.tensor.matmul(p1[:, ci * 512:ci * 512 + 500],
                                             s1, M2[:, c0:c0 + 500],
                                             start=True, stop=True)
                        pending_stt.append((p1, t1, g * UPG + u))
                        if len(pending_stt) > STT_DELAY:
                            emit_stt(*pending_stt.pop(0))
                        # ---- fe2 + av2, emitted 2 units late
                        pending_fe2.append((t1, g, u))
                        if len(pending_fe2) > FE2_DELAY:
                            emit_fe2(*pending_fe2.pop(0))

                    # ---- fn stage per pair of graphs, delayed 2 graphs so its
                    # PE/ACT instructions never wait on a fresh AV (in-order
                    # engine queues would stall the next graph's fe work).
                    if g % 2 == 1 and g >= 3:
                        _fn_pair(nc, g - 3, qbufs.pop((g - 3) // 2), xt,
                                 xnext if blk < 2 else None,
                                 ty, d, od, fn1avT, fn1xT, fn2T, fb1, fb2,
                                 ps1, ps2, y1pool, F32, BF16, AF, blk)
                while pending_stt:
                    emit_stt(*pending_stt.pop(0))
                while pending_fe2:
                    emit_fe2(*pending_fe2.pop(0))
                while pending_trees:
                    emit_tree(*pending_trees.pop(0))
                for g0 in (GPC - 2,):
                    _fn_pair(nc, g0, qbufs.pop(g0 // 2), xt,
                             xnext if blk < 2 else None,
                             ty, d, od, fn1avT, fn1xT, fn2T, fb1, fb2,
                             ps1, ps2, y1pool, F32, BF16, AF, blk)
                if blk < 2:
                    xt = xnext

            # ---------- final: sigmoid(mean over N) ----------
            red = xpool.tile([1, GPC], F32, tag="red")
            nc.vector.tensor_reduce(red[:], ty[:].rearrange("p (a b) -> p a b", a=GPC),
                                    axis=AX.X, op=ALU.add)
            osb = xpool.tile([1, GPC], F32, tag="osb")
            nc.scalar.activation(osb[:], red[:], AF.Sigmoid, scale=1.0 / N)
            nc.sync.dma_start(out_d.ap(), osb[:])

    nc.compile()
    return nc


def _host_prep(inputs):
    """Build per-core in_maps from full inputs."""
    x = np.asarray(inputs["x"], np.float32)          # [B, N, 2]

    bfl, nbf = _bf_pack_layout()
    f32l, nf32 = _f32_pack_layout()
    bfpack = np.zeros((128, nbf), np.float32)
    f32pack = np.zeros((128, nf32), np.float32)

    def put(name, arr):
        arr = np.asarray(arr, np.float32)
        for lay, pack in ((bfl, bfpack), (f32l, f32pack)):
            if name in lay:
                r, c, w = lay[name]
                assert arr.shape == (r, w), (name, arr.shape, (r, w))
                pack[0:r, c:c + w] = arr
                return
        raise KeyError(name)

    for i in range(3):
        d = NODE_SIZES[i]
        fe1w = np.asarray(inputs[f"fe1w{i}"], np.float32)   # [128, 2d+1]
        fe1b = np.asarray(inputs[f"fe1b{i}"], np.float32)
        fe2w = np.asarray(inputs[f"fe2w{i}"], np.float32)   # [128, 128]
        fe2b = np.asarray(inputs[f"fe2b{i}"], np.float32)
        fn1w = np.asarray(inputs[f"fn1w{i}"], np.float32)   # [128, 128+d]
        fn1b = np.asarray(inputs[f"fn1b{i}"], np.float32)
        fn2w = np.asarray(inputs[f"fn2w{i}"], np.float32)   # [od, 128]
        fn2b = np.asarray(inputs[f"fn2b{i}"], np.float32)
        if i == 0:
            perm = np.arange(d)
        else:
            # my x row order [y..., c0, c1] -> ref order [c0, c1, y...]
            perm = np.concatenate([np.arange(2, d), [0, 1]])
        W1a = fe1w[:, 0:d][:, perm].T                       # [d, 128]
        W1b = fe1w[:, d:2 * d][:, perm].T                   # [d, 128]
        s1 = np.concatenate(
            [W1a, W1b, fe1w[:, 2 * d].reshape(1, 128), fe1b.reshape(1, 128)], axis=0
        )
        put(f"s1_{i}", s1)
        put(f"fe2T{i}", fe2w.T)
        put(f"fn1xT{i}", fn1w[:, 128:][:, perm].T)
        put(f"fn1avT{i}", fn1w[:, :128].T)
        put(f"fn2T{i}", fn2w.T)
        put(f"b2_{i}", fe2b.reshape(128, 1))
        put(f"fb1_{i}", fn1b.reshape(128, 1))
        put(f"fb2_{i}", fn2b.reshape(FN2_OUT[i], 1))

    shared = {
        "bfpack": bfpack.astype(BF),
        "f32pack": f32pack,
    }

    in_maps = []
    for c in range(NCORES):
        xf = x[c * GPC:(c + 1) * GPC]                        # [16, 100, 2]
        xt0 = np.concatenate(
            [xf.transpose(2, 0, 1).reshape(2, COLS), np.ones((1, COLS), np.float32)],
            axis=0,
        )
        diff = xf[:, :, None, :] - xf[:, None, :, :]
        nrm = np.sqrt((diff * diff).sum(-1)).reshape(GPC, PAIRS)
        nrmo = np.empty((2 * GPC, PAIRS), np.float32)
        nrmo[0::2] = nrm
        nrmo[1::2] = 1.0
        m = dict(shared)
        m["xt0"] = xt0.astype(BF)
        m["nrmo"] = nrmo.astype(BF)
        in_maps.append(m)
    return in_maps


def kernel(**inputs):
    from concourse import bass_utils

    if "nc" not in _CACHE:
        _CACHE["nc"] = _build()
    nc = _CACHE["nc"]
    in_maps = _host_prep(inputs)
    res = bass_utils.run_bass_kernel_spmd(nc, in_maps, core_ids=list(range(NCORES)))
    out = np.concatenate(
        [np.asarray(res.results[c]["out"], np.float32).reshape(GPC, 1) for c in range(NCORES)],
        axis=0,
    )
    return out


# revision 34
# speedup vs baseline: 1.0385x; 1.0385x over previous
import sys
import numpy as np

for _p in ("/opt/trn_rl_repo", "/root/.axon_site/_ro/trn_rl_repo"):
    if _p not in sys.path:
        sys.path.append(_p)

import os
import ml_dtypes

BF = ml_dtypes.bfloat16

B, N, NODE, FE = 128, 100, 2, 128
NODE_SIZES = [2, 16, 32]
FN2_OUT = [14, 30, 1]
NCORES = 8
GPC = B // NCORES            # graphs per core = 16
COLS = GPC * N               # 1600
PAIRS = N * N                # 10000
UPG = 10                     # units (1000-col) per graph
CPB = GPC * UPG * 2          # 500-col chunks per block = 320

# Engine split fractions. GPSIMD cannot touch PSUM on real hardware, so the
# t1 lrelu (reads PSUM) runs on DVE ('D') or ACT ('A') at unit granularity;
# the SBUF-only tree adds are what Pool can take.
UPB = GPC * UPG                                      # units per block = 160
FRAC_A = float(os.environ.get("K_FRAC_A", 20 / 160))  # t1 units on ACT (rest DVE)
FRAC_L1D = float(os.environ.get("K_FRAC_L1D", 0.0))   # l1 units on DVE (rest Pool)
FRAC_L2P = float(os.environ.get("K_FRAC_L2P", 24 / 160))  # l2 units on Pool (rest DVE)
STT_DELAY = int(os.environ.get("K_STT_DELAY", 1))    # units
FE2_DELAY = int(os.environ.get("K_FE2_DELAY", 2))    # units
TREE_DELAY = int(os.environ.get("K_TREE_DELAY", 4))  # units
FN_SPREAD = int(os.environ.get("K_FN_SPREAD", 3))    # fn steps per unit


def _frac_assign(frac, mark, other, n=UPB):
    out = []
    acc = 0.0
    for _ in range(n):
        acc += frac
        if acc >= 1.0:
            out.append(mark)
            acc -= 1.0
        else:
            out.append(other)
    return out


def round_fp32r(a):
    u = np.ascontiguousarray(np.asarray(a, np.float32)).view(np.uint32)
    low = u & np.uint32(0xFFF)
    base = u & np.uint32(0xFFFFF000)
    add = ((low > 0x800) | ((low == 0x800) & (((u >> 12) & 1) == 1))).astype(np.uint32) << 12
    return (base + add).view(np.float32)


_CACHE = {}

# ---- packed constant layouts (shared between _build and _host_prep) ----
# bf16 pack: [128 rows, ...cols]: per block: s1 (2d+2 rows x 128), fe2T (128x128),
# fn1xT (d x 128)
def _bf_pack_layout():
    cols = {}
    c = 0
    for i in range(3):
        d = NODE_SIZES[i]
        od = FN2_OUT[i]
        cols[f"s1_{i}"] = (2 * d + 2, c, 128); c += 128
        cols[f"fe2T{i}"] = (128, c, 128); c += 128
        cols[f"fn1xT{i}"] = (d, c, 128); c += 128
        cols[f"fn1avT{i}"] = (128, c, 128); c += 128
        cols[f"fn2T{i}"] = (128, c, od); c += od
    return cols, c


def _f32_pack_layout():
    cols = {}
    c = 0
    for i in range(3):
        od = FN2_OUT[i]
        cols[f"b2_{i}"] = (128, c, 1); c += 1
        cols[f"fb1_{i}"] = (128, c, 1); c += 1
        cols[f"fb2_{i}"] = (od, c, 1); c += 1
    return cols, c


def _fn_pair_steps(nc, g0, Qbuf, xt, xnext, ty, d, od, fn1avT, fn1xT, fn2T,
                   fb1, fb2, ps2, y1pool, F32, BF16, AF, blk):
    """fn stage for graphs g0, g0+1 (200 node columns) as a list of emission
    closures so the PE matmuls can be interleaved with fe work instead of
    forming a serial block in PE's in-order queue. The av j-sum tail is folded
    into the fn1av matmul: Qbuf holds 25 partial-sum columns per node."""
    csl = slice(g0 * N, (g0 + 2) * N)
    state = {}
    steps = []

    def mk_start():
        state["pf"] = ps2.tile([128, 512], F32, tag="p2", name="pf")
        Qv = Qbuf[:].rearrange("p (a b) -> p a b", b=25)   # [128, 200, 25]
        state["Qv"] = Qv
        nc.tensor.matmul(state["pf"][:, 0:200], fn1avT, Qv[:, :, 0:1],
                         start=True, stop=False, skip_group_check=True)
    steps.append(mk_start)
    for c in range(1, 25):
        def mk_mm(c=c):
            nc.tensor.matmul(state["pf"][:, 0:200], fn1avT,
                             state["Qv"][:, :, c:c + 1],
                             start=False, stop=False, skip_group_check=True)
        steps.append(mk_mm)

    def mk_last():
        nc.tensor.matmul(state["pf"][:, 0:200], fn1xT, xt[0:d, csl],
                         start=False, stop=True, skip_group_check=True)
        y1 = y1pool.tile([128, 200], BF16, tag="y1")
        nc.scalar.activation(y1[:], state["pf"][:, 0:200], AF.Tanh, bias=fb1)
        pf2 = ps2.tile([od, 512], F32, tag="p2", name="pf2")
        nc.tensor.matmul(pf2[:, 0:200], fn2T, y1[:], start=True, stop=True)
        if blk < 2:
            nc.scalar.activation(xnext[0:od, csl], pf2[:, 0:200],
                                 AF.Tanh, bias=fb2)
        else:
            nc.scalar.activation(ty[:, csl], pf2[:, 0:200], AF.Tanh, bias=fb2)
    steps.append(mk_last)
    return steps


def _build():
    import concourse.bacc as bacc
    import concourse.mybir as mybir
    import concourse.tile as tile

    F32 = mybir.dt.float32
    F32R = mybir.dt.float32r
    BF16 = mybir.dt.bfloat16
    AF = mybir.ActivationFunctionType
    ALU = mybir.AluOpType
    AX = mybir.AxisListType

    nc = bacc.Bacc("TRN2", target_bir_lowering=False, debug=False, num_devices=NCORES)

    bfl, nbf = _bf_pack_layout()
    f32l, nf32 = _f32_pack_layout()

    din = {}
    din["xt0"] = nc.dram_tensor("xt0", [3, COLS], BF16, kind="ExternalInput")
    din["nrmo"] = nc.dram_tensor("nrmo", [2 * GPC, PAIRS], BF16, kind="ExternalInput")
    din["bfpack"] = nc.dram_tensor("bfpack", [128, nbf], BF16, kind="ExternalInput")
    din["f32pack"] = nc.dram_tensor("f32pack", [128, nf32], F32, kind="ExternalInput")
    out_d = nc.dram_tensor("out", [1, GPC], F32, kind="ExternalOutput")

    T1A = _frac_assign(FRAC_A, "A", "D")
    L1A = _frac_assign(FRAC_L1D, "D", "P")
    L2A = _frac_assign(FRAC_L2P, "P", "D")

    with tile.TileContext(nc) as tc:
        with (
            tc.tile_pool(name="const", bufs=1) as cpool,
            tc.tile_pool(name="xp", bufs=1) as xpool,
            tc.tile_pool(name="m2", bufs=3) as mpool,
            tc.tile_pool(name="rep", bufs=3) as rpool,
            tc.tile_pool(name="t1p", bufs=8) as tpool,
            tc.tile_pool(name="avp", bufs=8) as apool,
            tc.tile_pool(name="q1p", bufs=3) as q1pool,
            tc.tile_pool(name="qb", bufs=3) as qbpool,
            tc.tile_pool(name="y1p", bufs=2) as y1pool,
            tc.tile_pool(name="ps1", bufs=2, space="PSUM") as ps1,
            tc.tile_pool(name="ps2", bufs=2, space="PSUM") as ps2,
        ):
            XT0 = cpool.tile([3, COLS], BF16, tag="xt0", name="w_xt0")
            nc.sync.dma_start(XT0[:], din["xt0"].ap())
            BFP = cpool.tile([128, nbf], BF16, tag="bfp", name="w_bfp")
            nc.sync.dma_start(BFP[:], din["bfpack"].ap())
            # f32pack is only needed by the fn stage; load it after the
            # first graph's M2 DMAs are queued (SP queue is FIFO).
            F32P = cpool.tile([128, nf32], F32, tag="f32p", name="w_f32p")
            deferred_loads = [
                (F32P, din["f32pack"]),
            ]

            def Wv(name):
                if name in bfl:
                    r, c, w = bfl[name]
                    return BFP[0:r, c:c + w]
                r, c, w = f32l[name]
                return F32P[0:r, c:c + w]

            xt = XT0                        # [3, COLS] block-0 x (+ones row)
            ty = None
            for blk in range(3):
                d = NODE_SIZES[blk]
                od = FN2_OUT[blk]
                s1 = Wv(f"s1_{blk}")
                fe2T = Wv(f"fe2T{blk}")
                b2 = Wv(f"b2_{blk}")
                fn1avT, fn1xT = Wv(f"fn1avT{blk}"), Wv(f"fn1xT{blk}")
                fn2T = Wv(f"fn2T{blk}")
                fb1, fb2 = Wv(f"fb1_{blk}"), Wv(f"fb2_{blk}")
                # fe2/av2 are emitted 2 units late and tree (j-sum) work 4
                # units later still: every engine's in-order queue then only
                # sees instructions whose inputs were produced units ago, so
                # no engine parks waiting on a just-issued producer.
                pending_fe2 = []
                pending_trees = []
                pending_stt = []
                pending_fn = []
                qbufs = {}                   # graph-pair -> Qbuf tile

                def emit_stt(p1_, t1_, ku_):
                    p1v = p1_[:].rearrange("p (a b) -> p a b", b=512)[:, :, 0:500]
                    t1v = t1_[:].rearrange("p (a b) -> p a b", b=500)
                    if T1A[ku_ % UPB] == "A":
                        nc.scalar.activation(t1v, p1v, AF.Prelu,
                                             scale=1.0, alpha=0.2)
                    else:
                        nc.vector.scalar_tensor_tensor(t1v, p1v, 0.2, p1v,
                                                       ALU.mult, ALU.max)

                def emit_fe2(t1_, g_, u_):
                    p2 = ps2.tile([128, 1024], F32, tag="p2")
                    for ci in range(2):
                        nc.tensor.matmul(p2[:, ci * 512: ci * 512 + 500],
                                         fe2T, t1_[:, ci * 500:(ci + 1) * 500],
                                         start=True, stop=True)
                    p2v = p2[:].rearrange("p (a b) -> p a b", b=512)[:, :, 0:500]
                    av2 = apool.tile([128, 1000], BF16, tag="av2")
                    av2v = av2[:].rearrange("p (a b) -> p a b", b=500)
                    nc.scalar.activation(av2v, p2v, AF.Prelu,
                                         bias=b2, scale=1.0, alpha=0.2)
                    pending_trees.append((av2, g_, u_))
                    if len(pending_trees) > TREE_DELAY:
                        emit_tree(*pending_trees.pop(0))

                def emit_tree(av2_, g_, u_):
                    # two bf16 half-adds (SBUF-only, so Pool may take them);
                    # the remaining 25-way sum per node is folded into the
                    # fn1av matmul (see _fn_pair).
                    ku = g_ * UPG + u_
                    a3 = av2_[:].rearrange("p (a b) -> p a b", b=N)
                    q1 = q1pool.tile([128, 500], BF16, tag="q1")
                    q1v = q1[:].rearrange("p (a b) -> p a b", b=50)
                    e1 = nc.vector if L1A[ku % UPB] == "D" else nc.gpsimd
                    e1.tensor_tensor(q1v, a3[:, :, 0:50], a3[:, :, 50:100],
                                     ALU.add)
                    pr = g_ // 2
                    if pr not in qbufs:
                        qbufs[pr] = qbpool.tile([128, 5000], BF16, tag="qb", name=f"qb{pr % 3}")
                    base = (g_ % 2) * 2500 + u_ * 250
                    q2v = qbufs[pr][:, base:base + 250].rearrange(
                        "p (a b) -> p a b", b=25)
                    e2 = nc.gpsimd if L2A[ku % UPB] == "P" else nc.vector
                    e2.tensor_tensor(q2v, q1v[:, :, 0:25], q1v[:, :, 25:50],
                                     ALU.add)

                if blk < 2:
                    nd = NODE_SIZES[blk + 1]
                    nod = FN2_OUT[blk]
                    xnext = xpool.tile([nd + 1, COLS], BF16, tag=f"x{blk + 1}")
                    # coords + ones rows below the y rows, ready immediately
                    nc.sync.dma_start(xnext[nod:nod + 3, :], XT0[:, :])
                else:
                    ty = xpool.tile([1, COLS], F32, tag="ty")

                for g in range(GPC):
                    gsl = slice(g * N, (g + 1) * N)
                    # ---- build M2 = [x1; x2; nrm; ones]  [2d+2, PAIRS] bf16
                    rep4 = rpool.tile([d, 400], BF16, tag="rep4")
                    nc.sync.dma_start(
                        rep4[:].rearrange("p (a b) -> p a b", a=4),
                        xt[0:d, gsl].unsqueeze(1).broadcast_to([d, 4, N]),
                    )
                    M2 = mpool.tile([2 * d + 2, PAIRS], BF16, tag="m2")
                    nc.sync.dma_start(
                        M2[0:d, :].rearrange("p (a b) -> p a b", b=N),
                        xt[0:d, gsl].unsqueeze(2).broadcast_to([d, N, N]),
                    )
                    nc.sync.dma_start(
                        M2[d:2 * d, :].rearrange("p (a b) -> p a b", a=25),
                        rep4[:].unsqueeze(1).broadcast_to([d, 25, 400]),
                    )
                    nc.sync.dma_start(
                        M2[2 * d:2 * d + 2, :],
                        din["nrmo"].ap()[2 * g:2 * g + 2, :],
                    )
                    while deferred_loads:
                        tile_, dram_ = deferred_loads.pop(0)
                        nc.sync.dma_start(tile_[:], dram_.ap())

                    for u in range(UPG):
                        t1 = tpool.tile([128, 1000], BF16, tag="t1")
                        # ---- fe1 (bias baked in); the t1 lrelu is emitted
                        # late so no engine's queue head ever waits on a
                        # just-issued matmul.
                        p1 = ps1.tile([128, 1024], F32, tag="p1")
                        for ci in range(2):
                            c0 = u * 1000 + ci * 500
                            nc.tensor.matmul(p1[:, ci * 512:ci * 512 + 500],
                                             s1, M2[:, c0:c0 + 500],
                                             start=True, stop=True)
                        pending_stt.append((p1, t1, g * UPG + u))
                        if len(pending_stt) > STT_DELAY:
                            emit_stt(*pending_stt.pop(0))
                        # ---- fe2 + av2, emitted 2 units late
                        pending_fe2.append((t1, g, u))
                        if len(pending_fe2) > FE2_DELAY:
                            emit_fe2(*pending_fe2.pop(0))
                        # drip-feed deferred fn-pair matmuls between units
                        for _ in range(FN_SPREAD):
                            if pending_fn:
                                pending_fn.pop(0)()

                    # ---- fn stage per pair of graphs, delayed 2 graphs so its
                    # PE/ACT instructions never wait on a fresh AV (in-order
                    # engine queues would stall the next graph's fe work).
                    if g % 2 == 1 and g >= 3:
                        pending_fn.extend(_fn_pair_steps(
                            nc, g - 3, qbufs.pop((g - 3) // 2), xt,
                            xnext if blk < 2 else None,
                            ty, d, od, fn1avT, fn1xT, fn2T, fb1, fb2,
                            ps2, y1pool, F32, BF16, AF, blk))
                while pending_stt:
                    emit_stt(*pending_stt.pop(0))
                while pending_fe2:
                    emit_fe2(*pending_fe2.pop(0))
                while pending_trees:
                    emit_tree(*pending_trees.pop(0))
                while pending_fn:
                    pending_fn.pop(0)()
                for g0 in (GPC - 2,):
                    for step in _fn_pair_steps(
                            nc, g0, qbufs.pop(g0 // 2), xt,
                            xnext if blk < 2 else None,
                            ty, d, od, fn1avT, fn1xT, fn2T, fb1, fb2,
                            ps2, y1pool, F32, BF16, AF, blk):
                        step()
                if blk < 2:
                    xt = xnext

            # ---------- final: sigmoid(mean over N) ----------
            red = xpool.tile([1, GPC], F32, tag="red")
            nc.vector.tensor_reduce(red[:], ty[:].rearrange("p (a b) -> p a b", a=GPC),
                                    axis=AX.X, op=ALU.add)
            osb = xpool.tile([1, GPC], F32, tag="osb")
            nc.scalar.activation(osb[:], red[:], AF.Sigmoid, scale=1.0 / N)
            nc.sync.dma_start(out_d.ap(), osb[:])

    nc.compile()
    return nc


def _host_prep(inputs):
    """Build per-core in_maps from full inputs."""
    x = np.asarray(inputs["x"], np.float32)          # [B, N, 2]

    bfl, nbf = _bf_pack_layout()
    f32l, nf32 = _f32_pack_layout()
    bfpack = np.zeros((128, nbf), np.float32)
    f32pack = np.zeros((128, nf32), np.float32)

    def put(name, arr):
        arr = np.asarray(arr, np.float32)
        for lay, pack in ((bfl, bfpack), (f32l, f32pack)):
            if name in lay:
                r, c, w = lay[name]
                assert arr.shape == (r, w), (name, arr.shape, (r, w))
                pack[0:r, c:c + w] = arr
                return
        raise KeyError(name)

    for i in range(3):
        d = NODE_SIZES[i]
        fe1w = np.asarray(inputs[f"fe1w{i}"], np.float32)   # [128, 2d+1]
        fe1b = np.asarray(inputs[f"fe1b{i}"], np.float32)
        fe2w = np.asarray(inputs[f"fe2w{i}"], np.float32)   # [128, 128]
        fe2b = np.asarray(inputs[f"fe2b{i}"], np.float32)
        fn1w = np.asarray(inputs[f"fn1w{i}"], np.float32)   # [128, 128+d]
        fn1b = np.asarray(inputs[f"fn1b{i}"], np.float32)
        fn2w = np.asarray(inputs[f"fn2w{i}"], np.float32)   # [od, 128]
        fn2b = np.asarray(inputs[f"fn2b{i}"], np.float32)
        if i == 0:
            perm = np.arange(d)
        else:
            # my x row order [y..., c0, c1] -> ref order [c0, c1, y...]
            perm = np.concatenate([np.arange(2, d), [0, 1]])
        W1a = fe1w[:, 0:d][:, perm].T                       # [d, 128]
        W1b = fe1w[:, d:2 * d][:, perm].T                   # [d, 128]
        s1 = np.concatenate(
            [W1a, W1b, fe1w[:, 2 * d].reshape(1, 128), fe1b.reshape(1, 128)], axis=0
        )
        put(f"s1_{i}", s1)
        put(f"fe2T{i}", fe2w.T)
        put(f"fn1xT{i}", fn1w[:, 128:][:, perm].T)
        put(f"fn1avT{i}", fn1w[:, :128].T)
        put(f"fn2T{i}", fn2w.T)
        put(f"b2_{i}", fe2b.reshape(128, 1))
        put(f"fb1_{i}", fn1b.reshape(128, 1))
        put(f"fb2_{i}", fn2b.reshape(FN2_OUT[i], 1))

    shared = {
        "bfpack": bfpack.astype(BF),
        "f32pack": f32pack,
    }

    in_maps = []
    for c in range(NCORES):
        xf = x[c * GPC:(c + 1) * GPC]                        # [16, 100, 2]
        xt0 = np.concatenate(
            [xf.transpose(2, 0, 1).reshape(2, COLS), np.ones((1, COLS), np.float32)],
            axis=0,
        )
        diff = xf[:, :, None, :] - xf[:, None, :, :]
        nrm = np.sqrt((diff * diff).sum(-1)).reshape(GPC, PAIRS)
        nrmo = np.empty((2 * GPC, PAIRS), np.float32)
        nrmo[0::2] = nrm
        nrmo[1::2] = 1.0
        m = dict(shared)
        m["xt0"] = xt0.astype(BF)
        m["nrmo"] = nrmo.astype(BF)
        in_maps.append(m)
    return in_maps


def kernel(**inputs):
    from concourse import bass_utils

    if "nc" not in _CACHE:
        _CACHE["nc"] = _build()
    nc = _CACHE["nc"]
    in_maps = _host_prep(inputs)
    res = bass_utils.run_bass_kernel_spmd(nc, in_maps, core_ids=list(range(NCORES)))
    out = np.concatenate(
        [np.asarray(res.results[c]["out"], np.float32).reshape(GPC, 1) for c in range(NCORES)],
        axis=0,
    )
    return out


# revision 35
# speedup vs baseline: 1.2212x; 1.1760x over previous
import sys
import numpy as np

for _p in ("/opt/trn_rl_repo", "/root/.axon_site/_ro/trn_rl_repo"):
    if _p not in sys.path:
        sys.path.append(_p)

import os
import ml_dtypes

BF = ml_dtypes.bfloat16

B, N, NODE, FE = 128, 100, 2, 128
NODE_SIZES = [2, 16, 32]
FN2_OUT = [14, 30, 1]
NCORES = 8
GPC = B // NCORES            # graphs per core = 16
COLS = GPC * N               # 1600
PAIRS = N * N                # 10000
UPG = 10                     # units (1000-col) per graph
CPB = GPC * UPG * 2          # 500-col chunks per block = 320

# Engine split fractions. GPSIMD cannot touch PSUM on real hardware, so the
# t1 lrelu (reads PSUM) runs on DVE ('D') or ACT ('A') at unit granularity;
# the SBUF-only tree adds are what Pool can take.
UPB = GPC * UPG                                      # units per block = 160
FRAC_A = float(os.environ.get("K_FRAC_A", 8 / 160))  # t1 units on ACT (rest DVE)
FRAC_L1D = float(os.environ.get("K_FRAC_L1D", 0.0))   # l1 units on DVE (rest Pool)
FRAC_L2P = float(os.environ.get("K_FRAC_L2P", 24 / 160))  # l2 units on Pool (rest DVE)
STT_DELAY = int(os.environ.get("K_STT_DELAY", 1))    # units
FE2_DELAY = int(os.environ.get("K_FE2_DELAY", 2))    # units
TREE_DELAY = int(os.environ.get("K_TREE_DELAY", 4))  # units
FN_SPREAD = int(os.environ.get("K_FN_SPREAD", 1))    # fn tail steps per unit


def _frac_assign(frac, mark, other, n=UPB):
    out = []
    acc = 0.0
    for _ in range(n):
        acc += frac
        if acc >= 1.0:
            out.append(mark)
            acc -= 1.0
        else:
            out.append(other)
    return out


def round_fp32r(a):
    u = np.ascontiguousarray(np.asarray(a, np.float32)).view(np.uint32)
    low = u & np.uint32(0xFFF)
    base = u & np.uint32(0xFFFFF000)
    add = ((low > 0x800) | ((low == 0x800) & (((u >> 12) & 1) == 1))).astype(np.uint32) << 12
    return (base + add).view(np.float32)


_CACHE = {}

# ---- packed constant layouts (shared between _build and _host_prep) ----
# bf16 pack: [128 rows, ...cols]: per block: s1 (2d+2 rows x 128), fe2T (128x128),
# fn1xT (d x 128)
def _bf_pack_layout():
    cols = {}
    c = 0
    for i in range(3):
        d = NODE_SIZES[i]
        od = FN2_OUT[i]
        cols[f"s1_{i}"] = (2 * d + 2, c, 128); c += 128
        cols[f"fe2T{i}"] = (128, c, 128); c += 128
        cols[f"fn1xT{i}"] = (d, c, 128); c += 128
        cols[f"fn1avT{i}"] = (128, c, 128); c += 128
        cols[f"fn2T{i}"] = (128, c, od); c += od
    return cols, c


def _f32_pack_layout():
    cols = {}
    c = 0
    for i in range(3):
        od = FN2_OUT[i]
        cols[f"b2_{i}"] = (128, c, 1); c += 1
        cols[f"fb1_{i}"] = (128, c, 1); c += 1
        cols[f"fb2_{i}"] = (od, c, 1); c += 1
    return cols, c


def _fn_pair_steps(nc, g0, Qbuf, xt, xnext, ty, d, od, fn1avT, fn1xT, fn2T,
                   fb1, fb2, ps1, ps2, y1pool, F32, BF16, AF, blk):
    """fn stage for graphs g0, g0+1 (200 node columns). The av j-sum is folded
    into the fn1av matmul: Qbuf holds 50 half-summed columns per node, summed
    here via 50 accumulating matmuls (PE has slack; this removes the l2/reduce
    passes from DVE entirely). Emits the matmul burst inline; returns the tail
    (tanh -> fn2 -> tanh) as a closure to emit a unit later."""
    csl = slice(g0 * N, (g0 + 2) * N)
    pf = ps1.tile([128, 512], F32, tag="p1", name="pf")
    Qv = Qbuf[:].rearrange("p (a b) -> p a b", b=50)       # [128, 200, 50]
    for c in range(50):
        nc.tensor.matmul(pf[:, 0:200], fn1avT, Qv[:, :, c:c + 1],
                         start=(c == 0), stop=False, skip_group_check=True)
    nc.tensor.matmul(pf[:, 0:200], fn1xT, xt[0:d, csl],
                     start=False, stop=True, skip_group_check=True)

    def tail():
        y1 = y1pool.tile([128, 200], BF16, tag="y1")
        nc.scalar.activation(y1[:], pf[:, 0:200], AF.Tanh, bias=fb1)
        pf2 = ps2.tile([od, 512], F32, tag="p2", name="pf2")
        nc.tensor.matmul(pf2[:, 0:200], fn2T, y1[:], start=True, stop=True)
        if blk < 2:
            nc.scalar.activation(xnext[0:od, csl], pf2[:, 0:200],
                                 AF.Tanh, bias=fb2)
        else:
            nc.scalar.activation(ty[:, csl], pf2[:, 0:200], AF.Tanh, bias=fb2)
    return [tail]


def _build():
    import concourse.bacc as bacc
    import concourse.mybir as mybir
    import concourse.tile as tile

    F32 = mybir.dt.float32
    F32R = mybir.dt.float32r
    BF16 = mybir.dt.bfloat16
    AF = mybir.ActivationFunctionType
    ALU = mybir.AluOpType
    AX = mybir.AxisListType

    nc = bacc.Bacc("TRN2", target_bir_lowering=False, debug=False, num_devices=NCORES)

    bfl, nbf = _bf_pack_layout()
    f32l, nf32 = _f32_pack_layout()

    din = {}
    din["xt0"] = nc.dram_tensor("xt0", [3, COLS], BF16, kind="ExternalInput")
    din["nrmo"] = nc.dram_tensor("nrmo", [2 * GPC, PAIRS], BF16, kind="ExternalInput")
    din["bfpack"] = nc.dram_tensor("bfpack", [128, nbf], BF16, kind="ExternalInput")
    din["f32pack"] = nc.dram_tensor("f32pack", [128, nf32], F32, kind="ExternalInput")
    out_d = nc.dram_tensor("out", [1, GPC], F32, kind="ExternalOutput")

    T1A = _frac_assign(FRAC_A, "A", "D")
    L1A = _frac_assign(FRAC_L1D, "D", "P")
    L2A = _frac_assign(FRAC_L2P, "P", "D")

    with tile.TileContext(nc) as tc:
        with (
            tc.tile_pool(name="const", bufs=1) as cpool,
            tc.tile_pool(name="xp", bufs=1) as xpool,
            tc.tile_pool(name="m2", bufs=3) as mpool,
            tc.tile_pool(name="rep", bufs=3) as rpool,
            tc.tile_pool(name="t1p", bufs=8) as tpool,
            tc.tile_pool(name="avp", bufs=8) as apool,
            tc.tile_pool(name="q1p", bufs=3) as q1pool,
            tc.tile_pool(name="qb", bufs=3) as qbpool,
            tc.tile_pool(name="y1p", bufs=2) as y1pool,
            tc.tile_pool(name="ps1", bufs=2, space="PSUM") as ps1,
            tc.tile_pool(name="ps2", bufs=2, space="PSUM") as ps2,
        ):
            XT0 = cpool.tile([3, COLS], BF16, tag="xt0", name="w_xt0")
            nc.sync.dma_start(XT0[:], din["xt0"].ap())
            BFP = cpool.tile([128, nbf], BF16, tag="bfp", name="w_bfp")
            nc.sync.dma_start(BFP[:], din["bfpack"].ap())
            # f32pack is only needed by the fn stage; load it after the
            # first graph's M2 DMAs are queued (SP queue is FIFO).
            F32P = cpool.tile([128, nf32], F32, tag="f32p", name="w_f32p")
            deferred_loads = [
                (F32P, din["f32pack"]),
            ]

            def Wv(name):
                if name in bfl:
                    r, c, w = bfl[name]
                    return BFP[0:r, c:c + w]
                r, c, w = f32l[name]
                return F32P[0:r, c:c + w]

            xt = XT0                        # [3, COLS] block-0 x (+ones row)
            ty = None
            for blk in range(3):
                d = NODE_SIZES[blk]
                od = FN2_OUT[blk]
                s1 = Wv(f"s1_{blk}")
                fe2T = Wv(f"fe2T{blk}")
                b2 = Wv(f"b2_{blk}")
                fn1avT, fn1xT = Wv(f"fn1avT{blk}"), Wv(f"fn1xT{blk}")
                fn2T = Wv(f"fn2T{blk}")
                fb1, fb2 = Wv(f"fb1_{blk}"), Wv(f"fb2_{blk}")
                # fe2/av2 are emitted 2 units late and tree (j-sum) work 4
                # units later still: every engine's in-order queue then only
                # sees instructions whose inputs were produced units ago, so
                # no engine parks waiting on a just-issued producer.
                pending_fe2 = []
                pending_trees = []
                pending_stt = []
                pending_fn = []
                qbufs = {}                   # graph-pair -> Qbuf tile

                def emit_stt(p1_, t1_, ku_):
                    p1v = p1_[:].rearrange("p (a b) -> p a b", b=512)[:, :, 0:500]
                    t1v = t1_[:].rearrange("p (a b) -> p a b", b=500)
                    if T1A[ku_ % UPB] == "A":
                        nc.scalar.activation(t1v, p1v, AF.Prelu,
                                             scale=1.0, alpha=0.2)
                    else:
                        nc.vector.scalar_tensor_tensor(t1v, p1v, 0.2, p1v,
                                                       ALU.mult, ALU.max)

                def emit_fe2(t1_, g_, u_):
                    p2 = ps2.tile([128, 1024], F32, tag="p2")
                    for ci in range(2):
                        nc.tensor.matmul(p2[:, ci * 512: ci * 512 + 500],
                                         fe2T, t1_[:, ci * 500:(ci + 1) * 500],
                                         start=True, stop=True)
                    p2v = p2[:].rearrange("p (a b) -> p a b", b=512)[:, :, 0:500]
                    av2 = apool.tile([128, 1000], BF16, tag="av2")
                    av2v = av2[:].rearrange("p (a b) -> p a b", b=500)
                    nc.scalar.activation(av2v, p2v, AF.Prelu,
                                         bias=b2, scale=1.0, alpha=0.2)
                    pending_trees.append((av2, g_, u_))
                    if len(pending_trees) > TREE_DELAY:
                        emit_tree(*pending_trees.pop(0))

                def emit_tree(av2_, g_, u_):
                    # one bf16 half-add (SBUF-only, so Pool can own it); the
                    # remaining 50-way sum per node is folded into the fn1av
                    # matmul (see _fn_pair_steps).
                    ku = g_ * UPG + u_
                    a3 = av2_[:].rearrange("p (a b) -> p a b", b=N)
                    pr = g_ // 2
                    if pr not in qbufs:
                        qbufs[pr] = qbpool.tile([128, 10000], BF16, tag="qb", name=f"qb{pr % 3}")
                    base = (g_ % 2) * 5000 + u_ * 500
                    q1v = qbufs[pr][:, base:base + 500].rearrange(
                        "p (a b) -> p a b", b=50)
                    e1 = nc.vector if L1A[ku % UPB] == "D" else nc.gpsimd
                    e1.tensor_tensor(q1v, a3[:, :, 0:50], a3[:, :, 50:100],
                                     ALU.add)

                if blk < 2:
                    nd = NODE_SIZES[blk + 1]
                    nod = FN2_OUT[blk]
                    xnext = xpool.tile([nd + 1, COLS], BF16, tag=f"x{blk + 1}")
                    # coords + ones rows below the y rows, ready immediately
                    nc.sync.dma_start(xnext[nod:nod + 3, :], XT0[:, :])
                else:
                    ty = xpool.tile([1, COLS], F32, tag="ty")

                for g in range(GPC):
                    gsl = slice(g * N, (g + 1) * N)
                    # ---- build M2 = [x1; x2; nrm; ones]  [2d+2, PAIRS] bf16
                    rep4 = rpool.tile([d, 400], BF16, tag="rep4")
                    nc.sync.dma_start(
                        rep4[:].rearrange("p (a b) -> p a b", a=4),
                        xt[0:d, gsl].unsqueeze(1).broadcast_to([d, 4, N]),
                    )
                    M2 = mpool.tile([2 * d + 2, PAIRS], BF16, tag="m2")
                    nc.sync.dma_start(
                        M2[0:d, :].rearrange("p (a b) -> p a b", b=N),
                        xt[0:d, gsl].unsqueeze(2).broadcast_to([d, N, N]),
                    )
                    nc.sync.dma_start(
                        M2[d:2 * d, :].rearrange("p (a b) -> p a b", a=25),
                        rep4[:].unsqueeze(1).broadcast_to([d, 25, 400]),
                    )
                    nc.sync.dma_start(
                        M2[2 * d:2 * d + 2, :],
                        din["nrmo"].ap()[2 * g:2 * g + 2, :],
                    )
                    while deferred_loads:
                        tile_, dram_ = deferred_loads.pop(0)
                        nc.sync.dma_start(tile_[:], dram_.ap())

                    for u in range(UPG):
                        t1 = tpool.tile([128, 1000], BF16, tag="t1")
                        # ---- fe1 (bias baked in); the t1 lrelu is emitted
                        # late so no engine's queue head ever waits on a
                        # just-issued matmul.
                        p1 = ps1.tile([128, 1024], F32, tag="p1")
                        for ci in range(2):
                            c0 = u * 1000 + ci * 500
                            nc.tensor.matmul(p1[:, ci * 512:ci * 512 + 500],
                                             s1, M2[:, c0:c0 + 500],
                                             start=True, stop=True)
                        pending_stt.append((p1, t1, g * UPG + u))
                        if len(pending_stt) > STT_DELAY:
                            emit_stt(*pending_stt.pop(0))
                        # ---- fe2 + av2, emitted 2 units late
                        pending_fe2.append((t1, g, u))
                        if len(pending_fe2) > FE2_DELAY:
                            emit_fe2(*pending_fe2.pop(0))
                        # drip-feed deferred fn-pair matmuls between units
                        for _ in range(FN_SPREAD):
                            if pending_fn:
                                pending_fn.pop(0)()

                    # ---- fn stage per pair of graphs, delayed 2 graphs so its
                    # PE/ACT instructions never wait on a fresh AV (in-order
                    # engine queues would stall the next graph's fe work).
                    if g % 2 == 1 and g >= 3:
                        pending_fn.extend(_fn_pair_steps(
                            nc, g - 3, qbufs.pop((g - 3) // 2), xt,
                            xnext if blk < 2 else None,
                            ty, d, od, fn1avT, fn1xT, fn2T, fb1, fb2,
                            ps1, ps2, y1pool, F32, BF16, AF, blk))
                while pending_stt:
                    emit_stt(*pending_stt.pop(0))
                while pending_fe2:
                    emit_fe2(*pending_fe2.pop(0))
                while pending_trees:
                    emit_tree(*pending_trees.pop(0))
                while pending_fn:
                    pending_fn.pop(0)()
                for g0 in (GPC - 2,):
                    for step in _fn_pair_steps(
                            nc, g0, qbufs.pop(g0 // 2), xt,
                            xnext if blk < 2 else None,
                            ty, d, od, fn1avT, fn1xT, fn2T, fb1, fb2,
                            ps1, ps2, y1pool, F32, BF16, AF, blk):
                        step()
                if blk < 2:
                    xt = xnext

            # ---------- final: sigmoid(mean over N) ----------
            red = xpool.tile([1, GPC], F32, tag="red")
            nc.vector.tensor_reduce(red[:], ty[:].rearrange("p (a b) -> p a b", a=GPC),
                                    axis=AX.X, op=ALU.add)
            osb = xpool.tile([1, GPC], F32, tag="osb")
            nc.scalar.activation(osb[:], red[:], AF.Sigmoid, scale=1.0 / N)
            nc.sync.dma_start(out_d.ap(), osb[:])

    nc.compile()
    return nc


def _host_prep(inputs):
    """Build per-core in_maps from full inputs."""
    x = np.asarray(inputs["x"], np.float32)          # [B, N, 2]

    bfl, nbf = _bf_pack_layout()
    f32l, nf32 = _f32_pack_layout()
    bfpack = np.zeros((128, nbf), np.float32)
    f32pack = np.zeros((128, nf32), np.float32)

    def put(name, arr):
        arr = np.asarray(arr, np.float32)
        for lay, pack in ((bfl, bfpack), (f32l, f32pack)):
            if name in lay:
                r, c, w = lay[name]
                assert arr.shape == (r, w), (name, arr.shape, (r, w))
                pack[0:r, c:c + w] = arr
                return
        raise KeyError(name)

    for i in range(3):
        d = NODE_SIZES[i]
        fe1w = np.asarray(inputs[f"fe1w{i}"], np.float32)   # [128, 2d+1]
        fe1b = np.asarray(inputs[f"fe1b{i}"], np.float32)
        fe2w = np.asarray(inputs[f"fe2w{i}"], np.float32)   # [128, 128]
        fe2b = np.asarray(inputs[f"fe2b{i}"], np.float32)
        fn1w = np.asarray(inputs[f"fn1w{i}"], np.float32)   # [128, 128+d]
        fn1b = np.asarray(inputs[f"fn1b{i}"], np.float32)
        fn2w = np.asarray(inputs[f"fn2w{i}"], np.float32)   # [od, 128]
        fn2b = np.asarray(inputs[f"fn2b{i}"], np.float32)
        if i == 0:
            perm = np.arange(d)
        else:
            # my x row order [y..., c0, c1] -> ref order [c0, c1, y...]
            perm = np.concatenate([np.arange(2, d), [0, 1]])
        W1a = fe1w[:, 0:d][:, perm].T                       # [d, 128]
        W1b = fe1w[:, d:2 * d][:, perm].T                   # [d, 128]
        s1 = np.concatenate(
            [W1a, W1b, fe1w[:, 2 * d].reshape(1, 128), fe1b.reshape(1, 128)], axis=0
        )
        put(f"s1_{i}", s1)
        put(f"fe2T{i}", fe2w.T)
        put(f"fn1xT{i}", fn1w[:, 128:][:, perm].T)
        put(f"fn1avT{i}", fn1w[:, :128].T)
        put(f"fn2T{i}", fn2w.T)
        put(f"b2_{i}", fe2b.reshape(128, 1))
        put(f"fb1_{i}", fn1b.reshape(128, 1))
        put(f"fb2_{i}", fn2b.reshape(FN2_OUT[i], 1))

    shared = {
        "bfpack": bfpack.astype(BF),
        "f32pack": f32pack,
    }

    in_maps = []
    for c in range(NCORES):
        xf = x[c * GPC:(c + 1) * GPC]                        # [16, 100, 2]
        xt0 = np.concatenate(
            [xf.transpose(2, 0, 1).reshape(2, COLS), np.ones((1, COLS), np.float32)],
            axis=0,
        )
        diff = xf[:, :, None, :] - xf[:, None, :, :]
        nrm = np.sqrt((diff * diff).sum(-1)).reshape(GPC, PAIRS)
        nrmo = np.empty((2 * GPC, PAIRS), np.float32)
        nrmo[0::2] = nrm
        nrmo[1::2] = 1.0
        m = dict(shared)
        m["xt0"] = xt0.astype(BF)
        m["nrmo"] = nrmo.astype(BF)
        in_maps.append(m)
    return in_maps


def kernel(**inputs):
    from concourse import bass_utils

    if "nc" not in _CACHE:
        _CACHE["nc"] = _build()
    nc = _CACHE["nc"]
    in_maps = _host_prep(inputs)
    res = bass_utils.run_bass_kernel_spmd(nc, in_maps, core_ids=list(range(NCORES)))
    out = np.concatenate(
        [np.asarray(res.results[c]["out"], np.float32).reshape(GPC, 1) for c in range(NCORES)],
        axis=0,
    )
    return out
